# revision 12
# baseline (speedup 1.0000x reference)
"""Trainium2 Bass kernel for nn_LinearAttention (random-feature attention).

Reference computation (B=4, S=4096, D=U=R=256, fp32):
    Q = query @ Wq + bq                      [B,S,U]
    K = value @ Wk + bk                      [B,S,U]
    V = value @ Wv + bv                      [B,S,U]
    K_hat = cos(K @ Wr + br)                 [B,S,R]
    out = softmax(Q @ K_hat^T) @ V           [B,S,U]

Sharding: 8 cores, core c handles batch b=c//2, query-half h=c%2 (2048
queries). Each core needs the full key/value sequence of its batch.

Per-core algebraic restructurings (all exact up to fp rounding):
  * W_kr = (Wk @ Wr)/2pi, b_kr = (Wr^T bk + br)/2pi precomputed on device:
    K_hat chain runs directly off value^T (K projection eliminated).
  * cos(t) = 1 - 2 sin^2(pi frac(t/2pi)) and softmax is invariant to a
    per-query shift, so we store kh := 2 sin^2(.) and negate Q: the
    "1 -" pass disappears and scores shrink (std 7 vs 12 - exp safe).
  * V projection deferred through the attention matmul (associativity):
    out^T = Wv^T (value^T probs^T) + rowsum * bv.  The PV stage uses raw
    value tiles as stationary operands (V projection + copies eliminated);
    a tiny per-q-block Wv^T @ Z fixup restores the projection; bv is added
    during the PSUM->SBUF move in the output stage (softmax rows sum to 1).
  * Transposes use a bf16 identity (cost follows the moving operand =
    identity: 1 cycle/row instead of 2) with float32r-bitcast data, which
    the simulator moves exactly.
  * All high-volume matmul moving operands are float32r (1 cycle/row).
    fp32 tiles are bitcast to float32r at the matmul call (no copies).
  * Attention kt-loop software-pipelined: PE stream is sc(kt), Z(kt-1) so
    the PE never waits on the Act exp; previous q-block's output drain is
    emitted into the next q-block's early iterations.
"""
import sys

if "/opt/trn_rl_repo" not in sys.path:
    sys.path.insert(0, "/opt/trn_rl_repo")

import numpy as np
import concourse.bass as bass
import concourse.bacc as bacc
import concourse.tile as tile
from concourse import mybir
from concourse.bass_utils import run_bass_kernel_spmd
from concourse.masks import make_identity

FP = mybir.dt.float32
FR = mybir.dt.float32r
BF = mybir.dt.bfloat16
AF = mybir.ActivationFunctionType

P = 128          # partitions
B, S, DIM = 4, 4096, 256
SQ = S // 2      # queries per core
NC = 8           # cores
DC = DIM // P    # 2 chunks of the feature dims (d, u, r)
KT = S // P      # 32 key tiles
QB = 512         # q-block (psum bank = 512 fp32)
NQB = SQ // QB   # 4 q-blocks
ST = S // P      # 32 seq tiles for value
TPB = QB // P    # seq tiles per 512-block

INV2PI = float(1.0 / (2.0 * np.pi))
MAGIC = 12582912.0  # 1.5 * 2^23: fp32 round-to-nearest-int trick
SQRT2 = float(np.sqrt(2.0))


def build_kernel(nc: bass.Bass):
    ADD, SUB, MUL = (mybir.AluOpType.add, mybir.AluOpType.subtract,
                     mybir.AluOpType.mult)
    q_in = nc.dram_tensor("q_shard", [SQ, DIM], FP, kind="ExternalInput")
    v_in = nc.dram_tensor("v_full", [S, DIM], FP, kind="ExternalInput")
    w_q = nc.dram_tensor("Wq", [DIM, DIM], FP, kind="ExternalInput")
    w_k = nc.dram_tensor("Wk", [DIM, DIM], FP, kind="ExternalInput")
    w_v = nc.dram_tensor("Wv", [DIM, DIM], FP, kind="ExternalInput")
    w_r = nc.dram_tensor("Wr", [DIM, DIM], FP, kind="ExternalInput")
    b_q = nc.dram_tensor("bq", [DIM], FP, kind="ExternalInput")
    b_k = nc.dram_tensor("bk", [DIM], FP, kind="ExternalInput")
    b_v = nc.dram_tensor("bv", [DIM], FP, kind="ExternalInput")
    b_r = nc.dram_tensor("br", [DIM], FP, kind="ExternalInput")
    out = nc.dram_tensor("out", [SQ, DIM], FP, kind="ExternalOutput")

    with tile.TileContext(nc) as tc:
        with tc.tile_pool(name="singles", bufs=1) as singles, \
             tc.tile_pool(name="persist", bufs=1) as persist:
            ident = singles.tile([P, P], FR)
            make_identity(nc, ident)
            ones_col = singles.tile([P, 1], FP)
            nc.vector.memset(ones_col, 1.0)
            ones_row = singles.tile([1, QB], FP)
            nc.vector.memset(ones_row, 1.0)

            # weights straight into fp32 SBUF; bitcast to float32r at use
            wq_sb = singles.tile([P, DC, DIM], FP)
            nc.sync.dma_start(out=wq_sb,
                              in_=w_q.rearrange("(c p) u -> p c u", p=P))
            wv_sb = singles.tile([P, DC, DIM], FP)
            nc.sync.dma_start(out=wv_sb,
                              in_=w_v.rearrange("(c p) u -> p c u", p=P))
            wr_sb = singles.tile([P, DC, DIM], FP)
            nc.sync.dma_start(out=wr_sb,
                              in_=w_r.rearrange("(c p) u -> p c u", p=P))
            bq_sb = singles.tile([P, DC], FP)
            nc.sync.dma_start(out=bq_sb, in_=b_q.rearrange("(c p) -> p c", p=P))
            bk_sb = singles.tile([P, DC], FP)
            nc.sync.dma_start(out=bk_sb, in_=b_k.rearrange("(c p) -> p c", p=P))
            bv_sb = singles.tile([P, DC], FP)
            nc.sync.dma_start(out=bv_sb, in_=b_v.rearrange("(c p) -> p c", p=P))

            wkr_sb = singles.tile([P, DC, DIM], FP)   # (Wk @ Wr) / 2pi
            bkr_row = singles.tile([1, DIM], FP)      # (Wr^T bk + br) / 2pi

            # persistent stage outputs
            qT_p = persist.tile([P, DC, SQ], FP, tag="qT_proj")  # -(Q^T)
            kh_sb = persist.tile([P, DC, S], FP, tag="khat")     # 2sin^2 form
            v_nat = persist.tile([P, ST, DIM], FP, tag="v_nat")  # raw value

            # ---- precompute W_kr, b_kr on device ------------------------
            with tc.tile_pool(name="pre", bufs=2) as pre, \
                 tc.tile_pool(name="pre_ps", bufs=2, space="PSUM") as preps:
                wk_st = pre.tile([P, DC, DIM], FP, tag="wk")
                nc.sync.dma_start(out=wk_st,
                                  in_=w_k.rearrange("(c p) u -> p c u", p=P))
                wkT = pre.tile([P, DC, DIM], FP, tag="wkT")  # [u, (dc d)]
                for uc in range(DC):
                    tb = preps.tile([P, DIM], FR, tag="tr")
                    for dc in range(DC):
                        nc.tensor.matmul(
                            tb[:, dc * P:(dc + 1) * P],
                            wk_st[:, dc, uc * P:(uc + 1) * P].bitcast(FR),
                            ident, is_transpose=True,
                            start=(dc == 0), stop=(dc == DC - 1))
                    nc.vector.tensor_copy(wkT[:, uc, :], tb)
                for dc in range(DC):
                    psw = preps.tile([P, DIM], FP, tag="wkr")
                    for uc in range(DC):
                        nc.tensor.matmul(
                            psw, wkT[:, uc, dc * P:(dc + 1) * P].bitcast(FR),
                            wr_sb[:, uc, :].bitcast(FR),
                            start=(uc == 0), stop=(uc == DC - 1))
                    nc.vector.tensor_scalar(wkr_sb[:, dc, :], psw,
                                            INV2PI, 0.0, MUL, ADD)
                # b_kr as a row: [1,R] = bk^T @ Wr (lhsT = bk column)
                br_row = pre.tile([1, DIM], FP, tag="br_row")
                nc.sync.dma_start(out=br_row,
                                  in_=b_r.rearrange("(c u) -> c u", c=1))
                psb = preps.tile([1, DIM], FP, tag="bkr")
                for uc in range(DC):
                    nc.tensor.matmul(
                        psb, bk_sb[:, uc:uc + 1].bitcast(FR),
                        wr_sb[:, uc, :].bitcast(FR),
                        start=(uc == 0), stop=(uc == DC - 1))
                tmp_row = pre.tile([1, DIM], FP, tag="tmp_row")
                nc.vector.tensor_add(tmp_row, psb, br_row)
                nc.vector.tensor_scalar(bkr_row, tmp_row, INV2PI, 0.0,
                                        MUL, ADD)

            # ---- stage B/C: transposes, Q proj, K_hat -------------------
            copy_engines = (nc.vector, nc.scalar, nc.gpsimd)
            cp_i = 0
            with tc.tile_pool(name="qblk", bufs=2) as qblkp, \
                 tc.tile_pool(name="blocks", bufs=2) as blocks, \
                 tc.tile_pool(name="khtmp", bufs=2) as khtmp, \
                 tc.tile_pool(name="tps", bufs=2, space="PSUM") as tps, \
                 tc.tile_pool(name="pps", bufs=2, space="PSUM") as pps:

                def transpose_block(src, srow, blk_out):
                    # src[:, srow+t, dc*P:(dc+1)*P] tiles -> blk_out[:,dc,:]
                    nonlocal cp_i
                    for dc in range(DC):
                        bank = tps.tile([P, QB], FR, tag="tr")
                        for t in range(TPB):
                            nc.tensor.matmul(
                                bank[:, t * P:(t + 1) * P],
                                src[:, srow + t, dc * P:(dc + 1) * P].bitcast(FR),
                                ident, is_transpose=True,
                                start=(t == 0), stop=(t == TPB - 1))
                        eng = copy_engines[cp_i % 3]
                        cp_i += 1
                        if eng is nc.scalar:
                            eng.copy(blk_out[:, dc, :], bank)
                        else:
                            eng.tensor_copy(blk_out[:, dc, :], bank)

                def do_qb(qb):
                    qb_t = qblkp.tile([P, TPB, DIM], FP, tag="qin")
                    nc.sync.dma_start(
                        out=qb_t,
                        in_=q_in[qb * QB:(qb + 1) * QB, :].rearrange(
                            "(t p) d -> p t d", p=P))
                    qT_blk = blocks.tile([P, DC, QB], FP, tag="qT_blk")
                    transpose_block(qb_t, 0, qT_blk)
                    for uc in range(DC):
                        ps = pps.tile([P, QB], FP, tag="proj")
                        for dc in range(DC):
                            nc.tensor.matmul(
                                ps, wq_sb[:, dc, uc * P:(uc + 1) * P].bitcast(FR),
                                qT_blk[:, dc, :].bitcast(FR),
                                start=(dc == 0), stop=(dc == DC - 1))
                        # qT_p = -(Q^T): (ps + bq) * -1
                        nc.vector.tensor_scalar(
                            qT_p[:, uc, qb * QB:(qb + 1) * QB], ps,
                            bq_sb[:, uc:uc + 1], -1.0, ADD, MUL)

                for kb in range(S // QB):
                    nc.sync.dma_start(
                        out=v_nat[:, kb * TPB:(kb + 1) * TPB, :],
                        in_=v_in[kb * QB:(kb + 1) * QB, :].rearrange(
                            "(t p) d -> p t d", p=P))
                    if kb < NQB:
                        do_qb(kb)
                    vT_blk = blocks.tile([P, DC, QB], FP, tag="vT_blk")
                    transpose_block(v_nat, kb * TPB, vT_blk)

                    # kh = 2 sin^2(pi frac((K@Wr+br)/2pi)) (cos folded into
                    # negated Q via softmax shift invariance)
                    for rc in range(DC):
                        ps = pps.tile([P, QB], FP, tag="proj")
                        for dc in range(DC):
                            nc.tensor.matmul(
                                ps, wkr_sb[:, dc, rc * P:(rc + 1) * P].bitcast(FR),
                                vT_blk[:, dc, :].bitcast(FR),
                                start=(dc == 0), stop=False)
                        nc.tensor.matmul(
                            ps, bkr_row[0:1, rc * P:(rc + 1) * P].bitcast(FR),
                            ones_row.bitcast(FR), start=False, stop=True)
                        sl = slice(kb * QB, (kb + 1) * QB)
                        m_t = khtmp.tile([P, QB], FP, tag="kh_m")
                        nc.gpsimd.tensor_scalar(m_t, ps, MAGIC, MAGIC, ADD, SUB)
                        f_t = khtmp.tile([P, QB], FP, tag="kh_f")
                        nc.gpsimd.tensor_sub(f_t, ps, m_t)
                        s_t = khtmp.tile([P, QB], FP, tag="kh_s")
                        nc.scalar.activation(s_t, f_t, AF.Sin,
                                             scale=float(np.pi))
                        nc.scalar.activation(kh_sb[:, rc, sl], s_t, AF.Square,
                                             scale=SQRT2)

            # ---- stage D: attention ------------------------------------
            with tc.tile_pool(name="attn", bufs=3) as attn, \
                 tc.tile_pool(name="accp", bufs=2) as accp, \
                 tc.tile_pool(name="zsbp", bufs=2) as zsbp, \
                 tc.tile_pool(name="outp", bufs=2) as outp, \
                 tc.tile_pool(name="sc_ps", bufs=2, space="PSUM") as scp, \
                 tc.tile_pool(name="z_ps", bufs=2, space="PSUM") as zp, \
                 tc.tile_pool(name="tr_ps2", bufs=2, space="PSUM") as trp:

                def make_drain(qb, z_banks, acc0, acc1):
                    """Emit-later closure draining q-block qb's accumulators."""
                    state = {}

                    def piece0():
                        # PSUM Z -> SBUF as soon as the Z group stops
                        zsb = zsbp.tile([P, DC, QB], FP, tag="zsb", name="zsb")
                        for dc in range(DC):
                            nc.vector.tensor_copy(zsb[:, dc, :], z_banks[dc])
                        state["zsb"] = zsb

                    def piece1():
                        zsb = state["zsb"]
                        o2 = []
                        for uc in range(DC):
                            o2t = zp.tile([P, QB], FP, tag="o2", bufs=2,
                                          name=f"o2_{uc}")
                            for dc in range(DC):
                                nc.tensor.matmul(
                                    o2t,
                                    wv_sb[:, dc, uc * P:(uc + 1) * P].bitcast(FR),
                                    zsb[:, dc, :].bitcast(FR),
                                    start=(dc == 0), stop=(dc == DC - 1))
                            o2.append(o2t)
                        state["o2"] = o2
                        state["o_sb"] = outp.tile([P, TPB, DIM], FP,
                                                  tag="o_sb", name="o_sb")

                    def piece2(qt):
                        o2 = state["o2"]
                        o_sb = state["o_sb"]
                        qsl = slice(qt * P, (qt + 1) * P)
                        rs_t = trp.tile([P, P], FP, tag="ot_ps")
                        rs = rs_t[:, 0:1]
                        nc.tensor.matmul(rs, acc0[:, qsl], ones_col,
                                         start=True, stop=False)
                        nc.tensor.matmul(rs, acc1[:, qsl], ones_col,
                                         start=False, stop=True)
                        recip = outp.tile([P, 1], FP, tag="recip", bufs=4)
                        nc.vector.reciprocal(recip, rs)
                        for uh in range(DC):
                            ot = outp.tile([P, P], FR, tag="ot", bufs=2)
                            nc.vector.tensor_scalar_add(
                                ot, o2[uh][:, qsl], bv_sb[:, uh:uh + 1])
                            tp = trp.tile([P, P], FR, tag="ot_ps", name="tp")
                            nc.tensor.matmul(tp, ot, ident, is_transpose=True,
                                             start=True, stop=True)
                            nc.vector.tensor_scalar_mul(
                                o_sb[:, qt, uh * P:(uh + 1) * P],
                                tp.bitcast(FP), recip[:])

                    def piece3():
                        row0 = qb * QB
                        nc.sync.dma_start(
                            out=out[row0:row0 + QB, :].rearrange(
                                "(t p) u -> p t u", p=P),
                            in_=state["o_sb"])

                    return piece0, piece1, piece2, piece3

                pending = None
                for qb in range(NQB):
                    qs = slice(qb * QB, (qb + 1) * QB)
                    if pending is not None:
                        pending[0]()
                    z_banks = [zp.tile([P, QB], FP, tag="z", name=f"z{dc}")
                               for dc in range(DC)]
                    acc0 = accp.tile([P, QB], FP, tag="acc0")
                    acc1 = accp.tile([P, QB], FP, tag="acc1")
                    prev = None
                    for kt in range(KT):
                        sc = scp.tile([P, QB], FP, tag="sc")
                        for rc in range(DC):
                            nc.tensor.matmul(
                                sc, kh_sb[:, rc, kt * P:(kt + 1) * P].bitcast(FR),
                                qT_p[:, rc, qs].bitcast(FR),
                                start=(rc == 0), stop=(rc == DC - 1))
                        probs = attn.tile([P, QB], FP, tag="probs")
                        nc.scalar.activation(probs, sc, AF.Exp)
                        if kt == 0:
                            nc.vector.tensor_copy(acc0, probs)
                        elif kt == 1:
                            nc.gpsimd.tensor_copy(acc1, probs)
                        elif kt % 2 == 0:
                            nc.vector.tensor_add(acc0, acc0, probs)
                        else:
                            nc.gpsimd.tensor_add(acc1, acc1, probs)
                        if prev is not None:
                            pk, pp = prev
                            for dc in range(DC):
                                nc.tensor.matmul(
                                    z_banks[dc],
                                    v_nat[:, pk, dc * P:(dc + 1) * P].bitcast(FR),
                                    pp.bitcast(FR),
                                    start=(pk == 0), stop=False)
                        prev = (kt, probs)
                        if pending is not None:
                            if kt == 1:
                                pending[1]()
                            elif kt == 4:
                                pending[2](0)
                            elif kt == 5:
                                pending[2](1)
                            elif kt == 6:
                                pending[2](2)
                            elif kt == 7:
                                pending[2](3)
                                pending[3]()
                                pending = None
                    pk, pp = prev
                    for dc in range(DC):
                        nc.tensor.matmul(
                            z_banks[dc],
                            v_nat[:, pk, dc * P:(dc + 1) * P].bitcast(FR),
                            pp.bitcast(FR), start=False, stop=True)
                    pending = make_drain(qb, z_banks, acc0, acc1)
                # drain the last q-block
                pending[0]()
                pending[1]()
                for qt in range(TPB):
                    pending[2](qt)
                pending[3]()
    nc.finalize()
    return nc


_NC_CACHE = None


def _get_nc():
    global _NC_CACHE
    if _NC_CACHE is None:
        _NC_CACHE = build_kernel(bacc.Bacc(None, target_bir_lowering=False))
    return _NC_CACHE


def kernel(**inputs) -> np.ndarray:
    query = np.ascontiguousarray(np.asarray(inputs["query"], dtype=np.float32))
    value = np.ascontiguousarray(np.asarray(inputs["value"], dtype=np.float32))
    ws = {k: np.ascontiguousarray(np.asarray(inputs[k], dtype=np.float32))
          for k in ("Wq", "bq", "Wk", "bk", "Wv", "bv", "Wr", "br")}
    nc = _get_nc()
    in_maps = []
    for c in range(NC):
        b, h = c // 2, c % 2
        in_maps.append({
            "q_shard": np.ascontiguousarray(query[b, h * SQ:(h + 1) * SQ]),
            "v_full": value[b],
            **ws,
        })
    res = run_bass_kernel_spmd(nc, in_maps, core_ids=list(range(NC)))
    out = np.empty((B, S, DIM), np.float32)
    for c in range(NC):
        b, h = c // 2, c % 2
        out[b, h * SQ:(h + 1) * SQ] = res.results[c]["out"]
    return out
